# revision 31
# baseline (speedup 1.0000x reference)
"""Trainium2 Bass kernel for nn_Attention2D (B=8, C=256, H=W=32, 8 heads, d=32).

Strategy: data-parallel over batch, one batch element per NeuronCore (8 cores).

Per-core pipeline (n = H*W = 1024 tokens, head dim d = 32):
  phase 0: load x [256,1024] fp32 -> bf16; load host-prepped weights.
  qkv:     q = (scale*w_q) @ x, k = w_k @ x   ([256,1024] quad-major, bf16)
           vT = x^T @ w_v^T                   (8x [128,256] bf16 j-chunks)
  sim^T:   per unit (Q, ih, jc, h): matmul(lhsT=k slice [32,128], rhs=q slice
           [32,512]) -> PSUM chunk tiles [128,1024] (2 units); 4 heads run
           concurrently via row groups (K=32 packing). Softmax max-subtraction
           is skipped (logits ~N(0,0.8), measured max |sim| = 4.8).
  exp:     chunks alternate between the ACT engine (table exp, exact) and the
           DVE (Schraudolph bitcast exp: bits = round(x*128/ln2 + 127*128 - c)
           written as int16 aliasing the bf16 tile; ~±3% elementwise, washes
           out in the softmax average). This splits the elementwise roofline
           across two engines.
  AV:      per group (Q, ih): accumulate over (jc, h) into ONE quad PSUM tile
           main[32h:32h+32] (4-way column tiling, tile_position=(0,32h)) and a
           matching den quad tile via ones32 lhsT (denominator replicated).
  norm:    rc = reciprocal_approx_fast(den); out_all[Q][:, ih] = main * rc
           (full 128-row tiles, no junk rows).
  proj:    y = w_outT^T @ out_all + b_out, natural quad layout (no padding).
"""

import math

import numpy as np
import ml_dtypes

B, DIM, H, W = 8, 256, 32, 32
NUM_HEADS = 8
DIM_HEAD = 256
D = DIM_HEAD // NUM_HEADS          # 32 per-head dim
N = H * W                          # 1024 tokens
SCALE = (DIM_HEAD / NUM_HEADS) ** (-0.5)
NCORES = 8

_BF16 = ml_dtypes.bfloat16

_PROGRAM = None  # compiled Bass program cache (one per process)

# Schraudolph bf16 exp constants: bits = round(A16*x + B16), bitcast int16->bf16
A16 = 128.0 / math.log(2.0)
B16 = 127.0 * 128.0 - 5.58

# Groups in execution order: (Q, ih). Q = head quad, ih = token half.
GROUPS = [(0, 0), (1, 0), (0, 1), (1, 1)]

# Number of chunk pairs (of 32) whose second exp chunk runs on the DVE
# instead of ACT. Selected pairs are spread over [2, 31): the first two pairs
# keep the DVE free for startup evacs, the last pair keeps the tail on the
# (then idle) ACT engine.
N_DVE_PAIRS = 25


def _dve_chunk_ids(n_dve, lo=2, hi=31):
    """Chunk ids (hp=1 of selected pairs) spread evenly over [lo, hi)."""
    if n_dve <= 0:
        return set()
    total = hi - lo
    n_dve = min(n_dve, total)
    sel = {lo + i for i in range(total)
           if (i * n_dve) // total != ((i + 1) * n_dve) // total}
    return {2 * p + 1 for p in sel}


def build_kernel_body(tc, y_ap, x_ap, wqkvT_ap, woutT_ap, bout_ap):
    """Emit the per-core attention program into TileContext tc.

    DRAM tensors:
      x_ap:     [256, 1024] fp32   (one batch element, channels x tokens)
      wqkvT_ap: [256, 768]  bf16   (w_qkv^T, q-part pre-scaled by SCALE)
      woutT_ap: [256, 256]  bf16   (w_out^T, natural layout)
      bout_ap:  [256, 1]    fp32
      y_ap:     [256, 1024] fp32 out
    """
    from contextlib import ExitStack
    from concourse import mybir

    nc = tc.nc
    f32 = mybir.dt.float32
    bf16 = mybir.dt.bfloat16
    i16 = mybir.dt.int16

    dve_chunks = _dve_chunk_ids(N_DVE_PAIRS)

    with ExitStack() as ctx:
        singles = ctx.enter_context(tc.tile_pool(name="singles", bufs=1))
        evac = ctx.enter_context(tc.tile_pool(name="evac", bufs=2))
        exp_pool = ctx.enter_context(tc.tile_pool(name="exp", bufs=12))
        rc_pool = ctx.enter_context(tc.tile_pool(name="rc", bufs=2))
        sim_psum = ctx.enter_context(tc.tile_pool(name="simp", bufs=3, space="PSUM"))
        acc_psum = ctx.enter_context(tc.tile_pool(name="accp", bufs=2, space="PSUM"))

        # ---- phase 0: loads + constant prep ----
        # x arrives pre-converted to bf16 (host-side). The nh0 halves and the
        # qkv weights go out first, split across the two HWDGE queues, so the
        # first qkv matmul can start as early as possible.
        ones32 = singles.tile([128, 32], bf16, tag="ones32")
        nc.gpsimd.memset(ones32, 1.0)
        junk = singles.tile([128, 512], bf16, tag="junk")
        nc.gpsimd.memset(junk, 0.0)

        xb = [singles.tile([128, N], bf16, tag=f"xb_{c}", name=f"xb_{c}")
              for c in range(2)]
        wq = [singles.tile([128, 768], bf16, tag=f"wq_{c}", name=f"wq_{c}")
              for c in range(2)]
        for c in range(2):
            eng = nc.sync if c == 0 else nc.scalar
            eng.dma_start(out=xb[c][:, 0:512],
                          in_=x_ap[c * 128:(c + 1) * 128, 0:512])
        for c in range(2):
            eng = nc.sync if c == 0 else nc.scalar
            eng.dma_start(out=wq[c], in_=wqkvT_ap[c * 128:(c + 1) * 128, :])
        for c in range(2):
            eng = nc.sync if c == 0 else nc.scalar
            eng.dma_start(out=xb[c][:, 512:1024],
                          in_=x_ap[c * 128:(c + 1) * 128, 512:1024])

        # PE warm-up: throwaway matmuls during the DMA wait flip the HAM
        # clock gate to 8/8 before the real stream begins.
        warm_ps = acc_psum.tile([128, 512], f32, tag="acc", name="warm")
        for i in range(9):
            nc.tensor.matmul(warm_ps[0:32, :], ones32, junk,
                             start=(i == 0), stop=(i == 8))

        wo = []
        for t in range(2):
            tw = singles.tile([128, 256], bf16, tag=f"wo_{t}")
            nc.sync.dma_start(out=tw, in_=woutT_ap[t * 128:(t + 1) * 128, :])
            wo.append(tw)
        bias = []
        for oc in range(2):
            tb = singles.tile([128, 1], f32, tag=f"bias_{oc}")
            nc.scalar.dma_start(out=tb, in_=bout_ap[oc * 128:(oc + 1) * 128, :])
            bias.append(tb)

        # out_all: final-GEMM rhs in natural quad layout: out_all[Q] row
        # 32h + dd = head (4Q+h) dim dd.
        out_all = []
        for q in range(2):
            ta = singles.tile([128, N], bf16, tag=f"out_all_{q}")
            out_all.append(ta)

        qb = [None, None]
        kb = [None, None]

        def acc_tile(shape, name):
            return acc_psum.tile(shape, f32, tag="acc", name=name)

        def tmp_tile(shape, name):
            return acc_psum.tile(shape, f32, tag="acc", name=name)

        qk_dst = {}

        def emit_qkv_half(oc, nh):
            """o-chunks 0,1 -> q quads 0,1; o-chunks 2,3 -> k quads 0,1."""
            if oc not in qk_dst:
                qk_dst[oc] = singles.tile([128, N], bf16, tag=f"qk_{oc}",
                                          name=f"qk_{oc}")
                if oc < 2:
                    qb[oc] = qk_dst[oc]
                else:
                    kb[oc - 2] = qk_dst[oc]
            dst = qk_dst[oc]
            ps = tmp_tile([128, 512], f"qkvp{oc}_{nh}")
            for kc in range(2):
                nc.tensor.matmul(
                    ps,
                    wq[kc][:, oc * 128:(oc + 1) * 128],
                    xb[kc][:, nh * 512:(nh + 1) * 512],
                    start=(kc == 0),
                    stop=(kc == 1),
                )
            nc.vector.tensor_copy(out=dst[:, nh * 512:(nh + 1) * 512], in_=ps)

        vt = [None] * 8

        def emit_vt(jc):
            ps = tmp_tile([128, 256], f"vtps_{jc}")
            for kc in range(2):
                nc.tensor.matmul(
                    ps,
                    xb[kc][:, jc * 128:(jc + 1) * 128],
                    wq[kc][:, 512:768],
                    start=(kc == 0),
                    stop=(kc == 1),
                )
            dst = singles.tile([128, 256], bf16, tag=f"vt_{jc}")
            nc.vector.tensor_copy(out=dst, in_=ps)
            vt[jc] = dst

        # Only the halves the first sim pairs need; the rest is interleaved
        # into the early stream.
        emit_qkv_half(0, 0)
        emit_qkv_half(2, 0)

        # ---- main stream: chunk-pair pipeline with AV lag ----
        # A pair = (hp=0, hp=1) of one (group, jc): its 4 sim matmuls are
        # adjacent in the PE queue (4-way row tiling), its 4 main and 4 den
        # matmuls likewise (4-way column tiling). AV for pair p is emitted
        # after the sims of pair p+AV_LAG_PAIRS so the in-order PE queue never
        # head-of-line blocks on an unfinished exp.
        AV_LAG_PAIRS = 2
        pairs = []   # (g, Q, ih, jc)
        for g, (Q, ih) in enumerate(GROUPS):
            for jc in range(8):
                pairs.append((g, Q, ih, jc))
        n_pairs = len(pairs)
        mains = {}
        dens = {}
        exps = [None] * (2 * n_pairs)

        def emit_sim(c):
            p = c // 2
            hp = c % 2
            g, Q, ih, jc = pairs[p]
            heads = (2 * hp, 2 * hp + 1)
            ps = sim_psum.tile([128, 1024], f32, tag="sim", name=f"sim_{c}")
            et = exp_pool.tile([128, 1024], bf16, tag="exp", name=f"exp_{c}")
            exps[c] = et
            for s, h in enumerate(heads):
                tp = (96, 0) if h == 3 else None
                nc.tensor.matmul(
                    ps[:, s * 512:(s + 1) * 512],
                    kb[Q][32 * h:32 * (h + 1), jc * 128:(jc + 1) * 128],
                    qb[Q][32 * h:32 * (h + 1), ih * 512:(ih + 1) * 512],
                    start=True,
                    stop=True,
                    tile_position=tp,
                )
            return ps

        def emit_exp(c, ps):
            if c in dve_chunks:
                nc.vector.tensor_scalar(
                    out=exps[c].bitcast(i16),
                    in0=ps,
                    scalar1=A16,
                    scalar2=B16,
                    op0=mybir.AluOpType.mult,
                    op1=mybir.AluOpType.add,
                )
            else:
                nc.scalar.activation(
                    out=exps[c],
                    in_=ps,
                    func=mybir.ActivationFunctionType.Exp,
                )

        def emit_av_main(p):
            g, Q, ih, jc = pairs[p]
            if jc == 0:
                mains[g] = acc_tile([128, 512], f"main_{g}")
                dens[g] = acc_tile([128, 512], f"den_{g}")
            main = mains[g]
            st, sp = (jc == 0), (jc == 7)
            for h in range(4):
                rhs = exps[2 * p + h // 2][:, (h % 2) * 512:(h % 2 + 1) * 512]
                nc.tensor.matmul(
                    main[32 * h:32 * (h + 1), :],
                    vt[jc][:, 32 * (4 * Q + h):32 * (4 * Q + h) + 32],
                    rhs, start=st, stop=sp,
                    tile_position=(0, 32 * h))

        pending = []   # deferred emission closures, drained one per pair

        def emit_av_den(p):
            g, Q, ih, jc = pairs[p]
            den = dens[g]
            st, sp = (jc == 0), (jc == 7)
            for h in range(4):
                rhs = exps[2 * p + h // 2][:, (h % 2) * 512:(h % 2 + 1) * 512]
                nc.tensor.matmul(
                    den[32 * h:32 * (h + 1), :],
                    ones32,
                    rhs, start=st, stop=sp,
                    tile_position=(0, 32 * h))
            if jc == 7:
                rc = rc_pool.tile([128, 512], f32, tag="rc")
                nc.vector.reciprocal_approx_fast(out=rc, in_=dens[g])
                nc.vector.tensor_mul(
                    out=out_all[Q][:, ih * 512:(ih + 1) * 512],
                    in0=mains[g], in1=rc)
                if g == 1:
                    # allocate the proj psums now (so they take over the
                    # group-0/1 banks), but defer the matmuls one pair so the
                    # in-order PE queue never waits on the mul just issued.
                    prj = [tmp_tile([128, 512], f"proj0_{i}")
                           for i in range(2)]
                    pending.append(lambda: emit_proj(0, prj))

        def emit_proj(nh, ps_pair):
            for oc in range(2):
                ps = ps_pair[oc]
                for t in range(2):
                    nc.tensor.matmul(
                        ps,
                        wo[t][:, oc * 128:(oc + 1) * 128],
                        out_all[t][:, nh * 512:(nh + 1) * 512],
                        start=(t == 0),
                        stop=(t == 1),
                    )
                ys = evac.tile([128, 512], f32, tag="y")
                nc.scalar.activation(out=ys, in_=ps,
                                     func=mybir.ActivationFunctionType.Identity,
                                     bias=bias[oc])
                eng = nc.sync if oc == 0 else nc.scalar
                eng.dma_start(
                    out=y_ap[oc * 128:(oc + 1) * 128, nh * 512:(nh + 1) * 512],
                    in_=ys,
                )

        for p in range(n_pairs):
            g, Q, ih, jc = pairs[p]
            ps0 = emit_sim(2 * p)
            emit_exp(2 * p, ps0)
            ps1 = emit_sim(2 * p + 1)
            emit_exp(2 * p + 1, ps1)
            # interleave deferred prep work into the early stream
            if g == 0:
                emit_vt(jc)
                if jc == 0:
                    emit_qkv_half(0, 1)   # q quad0, token half 1
                elif jc == 1:
                    emit_qkv_half(2, 1)   # k quad0, token half 1
                elif jc == 2:
                    emit_qkv_half(1, 0)
                    emit_qkv_half(1, 1)   # q quad1
                elif jc == 3:
                    emit_qkv_half(3, 0)
                    emit_qkv_half(3, 1)   # k quad1
            elif pending:
                pending.pop(0)()
            if p >= AV_LAG_PAIRS:
                emit_av_main(p - AV_LAG_PAIRS)
                emit_av_den(p - AV_LAG_PAIRS)
        for p in range(n_pairs - AV_LAG_PAIRS, n_pairs):
            emit_av_main(p)
            emit_av_den(p)

        prj1 = [tmp_tile([128, 512], f"proj1_{i}") for i in range(2)]
        emit_proj(1, prj1)


def _prep_weights(w_qkv, w_out, b_out):
    """Host-side weight preparation (numpy)."""
    wq = w_qkv.astype(np.float32).copy()
    wq[0:DIM_HEAD] *= SCALE                      # fold softmax scale into w_q
    wqkvT = np.ascontiguousarray(wq.T).astype(_BF16)           # [256, 768]
    woutT = np.ascontiguousarray(
        w_out.astype(np.float32).T).astype(_BF16)              # [256, 256]
    bout = b_out.astype(np.float32).reshape(DIM, 1)            # [256, 1]
    return wqkvT, woutT, bout


def _strip_redundant_pe_waits(nc):
    """Drop transitively-implied sem waits from PE instructions.

    Walrus allows only one sync-wait command on a Matmult. Tile's semaphore
    pass is not transitively minimal: the first matmul writing a recycled
    PSUM slot waits both on the engine op that freed the slot AND on a
    tick that the freeing op itself already waited for. Strip wait W2 from a
    PE instruction when another wait W1 on it is served by an instruction
    that itself waited for W2's semaphore to reach at least W2's value.
    """
    for f in nc.m.functions:
        for blk in f.blocks:
            insts = list(blk.instructions)
            cum = {}
            served_by = {}  # (sem_name, cum_value) -> inst
            for ins in insts:
                if ins.sync_info is None:
                    continue
                for up in ins.sync_info.on_update:
                    if up.update_mode != "sem-inc":
                        continue
                    c = cum.get(up.ant_name, 0) + up.update_value
                    cum[up.ant_name] = c
                    served_by[(up.ant_name, c)] = ins

            def server_of(w):
                for v in range(w.wait_value, w.wait_value + 16):
                    srv = served_by.get((w.ant_name, v))
                    if srv is not None:
                        return srv
                return None

            def implied(w1, w2):
                srv = server_of(w1)
                if srv is None or srv.sync_info is None:
                    return False
                for w in srv.sync_info.on_wait:
                    if (w.ant_name == w2.ant_name
                            and w.wait_mode == "sem-ge-imm"
                            and w.wait_value >= w2.wait_value):
                        return True
                return False

            pe_idx = [k for k, ins in enumerate(insts)
                      if str(ins.engine) in ("EngineType.PE", "PE")]
            pe_pos = {k: p for p, k in enumerate(pe_idx)}

            for k, ins in enumerate(insts):
                if k not in pe_pos:
                    continue
                si = ins.sync_info
                if si is None:
                    continue
                waits = list(si.on_wait)
                while len(waits) > 1:
                    drop = None
                    for w2 in waits:
                        if w2.wait_mode != "sem-ge-imm":
                            continue
                        for w1 in waits:
                            if w1 is w2 or w1.wait_mode != "sem-ge-imm":
                                continue
                            if implied(w1, w2):
                                drop = w2
                                break
                        if drop is not None:
                            break
                    if drop is None:
                        # Move an excess wait onto the immediately preceding
                        # LDWEIGHTS. Same engine + in-order queue preserves
                        # the ordering; LDWEIGHTS never updates a semaphore,
                        # so nothing can wait on it and no cycle can form.
                        import bass_rust
                        p = pe_pos[k] - 1
                        carrier = insts[pe_idx[p]] if p >= 0 else None
                        if (carrier is None
                                or carrier.__class__.__name__ != "InstLdweights"
                                or (carrier.sync_info is not None
                                    and len(carrier.sync_info.on_wait) > 0)):
                            break
                        # keep the cross-engine (exp producer) wait on the MM
                        w2 = next((w for w in waits
                                   if w.ant_name.startswith("PE")), waits[-1])
                        carrier.sync_info = bass_rust.SyncInfo(
                            on_wait=[w2], on_update=[])
                        drop = w2
                    waits = [w for w in waits if w is not drop]
                if len(waits) != len(si.on_wait):
                    si.on_wait = waits
                if len(waits) > 1:
                    print(f"WARNING: {ins.name} still has {len(waits)} waits")


def _build_program():
    global _PROGRAM
    if _PROGRAM is not None:
        return _PROGRAM
    import concourse.tile as tile
    from concourse import bacc, mybir

    nc = bacc.Bacc("TRN2", target_bir_lowering=False, debug=False,
                   num_devices=NCORES)
    x_ap = nc.dram_tensor("x", [DIM, N], mybir.dt.bfloat16,
                          kind="ExternalInput").ap()
    wqkvT_ap = nc.dram_tensor("wqkvT", [DIM, 3 * DIM_HEAD], mybir.dt.bfloat16,
                              kind="ExternalInput").ap()
    woutT_ap = nc.dram_tensor("woutT", [DIM, DIM], mybir.dt.bfloat16,
                              kind="ExternalInput").ap()
    bout_ap = nc.dram_tensor("bout", [DIM, 1], mybir.dt.float32,
                             kind="ExternalInput").ap()
    y_ap = nc.dram_tensor("y", [DIM, N], mybir.dt.float32,
                          kind="ExternalOutput").ap()
    with tile.TileContext(nc) as tc:
        build_kernel_body(tc, y_ap, x_ap, wqkvT_ap, woutT_ap, bout_ap)
    _strip_redundant_pe_waits(nc)
    nc.compile()
    _PROGRAM = nc
    return nc


def kernel(x, w_qkv, w_out, b_out, trace=False):
    """Full-input entry point: shard over batch, run on 8 cores, gather."""
    from concourse import bass_utils

    nc = _build_program()
    wqkvT, woutT, bout = _prep_weights(w_qkv, w_out, b_out)
    in_maps = []
    for b in range(B):
        in_maps.append({
            "x": np.ascontiguousarray(
                np.asarray(x[b], dtype=np.float32).reshape(DIM, N)
            ).astype(_BF16),
            "wqkvT": wqkvT,
            "woutT": woutT,
            "bout": bout,
        })
    res = bass_utils.run_bass_kernel_spmd(
        nc, in_maps, core_ids=list(range(NCORES)), trace=trace)
    y = np.stack([res.results[b]["y"].reshape(DIM, H, W) for b in range(B)])
    kernel.last_results = res
    return y
